# revision 18
# baseline (speedup 1.0000x reference)
"""Trainium2 Bass kernel for dynamic low-pass filter decomposition, v4.

Module: global-avg-pool -> 1x1 conv -> BN (inference) -> softmax over 3x3
taps gives a per-(sample, group) 3x3 kernel; applied as a reflect-padded
depthwise conv over x; returns (low, x - low).

Sharding: data-parallel over batch n=8 across 8 NeuronCores (1 sample/core).

v4 layout ("row-band" partitioning; all device I/O in bf16, host permutes
x to [row][chan][w] so every DMA is a flat large-burst AP):

  The image's 192 rows are processed in 14 bands of 14 rows.  Within a
  band, partition p = i*8 + g where i = row-in-band (0..13) and g =
  channel group (0..7); partitions 112..119 hold the row above the band,
  120..127 the row below (reflection at the image edge resolved at DMA
  time).  The free dim is (c_sub 8) x (w 192) = 1536 per partition.

  With rows on partitions, the THREE vertical taps of the 3x3 kernel for
  one horizontal shift dj collapse into ONE matmul with a block-banded
  stationary S_dj[q, p] = w[g, di, dj] at q = src-row(p, di): the
  TensorEngine sums the vertical taps in-array.  3 matmuls per 512-col
  chunk instead of 9; only 3 stationaries total, built on-device by DVE
  from constant wiring patterns (E) row-scaled by the softmax weights.

  w=0/191 columns (horizontal reflect) are recomputed by one extra tiny
  N=16 matmul per (band, dj) into the 4th PSUM bank of the band's acc
  tile; ACT scatters them over the wrong values after the main copy.

  Pooled means: per band, a log2 tree of bf16 tensor_tensor adds folds w
  192->12 (DVE), then one matmul with the band's partial sums AS THE
  STATIONARY against a group-mask moving operand accumulates the
  cross-partition (row) sums into a [96, 8] PSUM tile; 8 tiny per-group
  matmuls against the BN-folded 1x1-conv weights produce the 72 logits.
"""
import sys
import os

sys.path.insert(0, "/opt/trn_rl_repo")

import numpy as np
import ml_dtypes
from contextlib import ExitStack

import concourse.bass as bass
import concourse.tile as tile
from concourse import bacc, mybir
from concourse.bass_utils import run_bass_kernel_spmd

dt = mybir.dt
f32 = dt.float32
bf16 = dt.bfloat16

KS = 3
GROUP = 8
IC = 64
BN_EPS = 1e-5
N = 8
H = W = 192
CW = 8 * W              # free elems per partition (8 chans x 192 cols)
PAD = 2                 # front/back pad elems (4B alignment + shift room)
BR = 14                 # rows per band
NBANDS = 14             # 14 * 14 = 196 >= 192
CH = 512
ROWSTRIDE = IC * W      # 12288 elems per image row in [r][c][w] layout


def _band_rows(t):
    """(first output row, n output rows) of band t."""
    r0 = BR * t
    return r0, min(BR, H - r0)


def _build_program():
    nc = bacc.Bacc("TRN2", target_bir_lowering=False, debug=False,
                   num_devices=N)

    x_d = nc.dram_tensor("x", [H, IC, W], bf16, kind="ExternalInput")
    e_d = [nc.dram_tensor(f"epat{di}", [128, 128], bf16,
                          kind="ExternalInput") for di in range(3)]
    hv_d = nc.dram_tensor("hv4", [8, 128], f32, kind="ExternalInput")
    gm_d = nc.dram_tensor("gmask", [128, 8], f32, kind="ExternalInput")
    at_d = nc.dram_tensor("at96", [96, 576], f32, kind="ExternalInput")
    b_d = nc.dram_tensor("b72", [72, 1], f32, kind="ExternalInput")
    r9_d = nc.dram_tensor("r9", [72, 9], f32, kind="ExternalInput")
    g_d = nc.dram_tensor("g728", [72, 8], f32, kind="ExternalInput")
    low_d = nc.dram_tensor("low", [H, IC, W], bf16, kind="ExternalOutput")
    high_d = nc.dram_tensor("high", [H, IC, W], bf16, kind="ExternalOutput")

    xd = x_d.ap().tensor

    def band_main_ap(t):
        """DRAM AP for band t's valid rows: partition (i, g) = i*8+g <-
        row r0+i, chans 8g..8g+8.  Since ROWSTRIDE = 8*CW this is a FLAT
        2-dim AP (partition stride CW) -- the shape the DMA engines spray
        across all 16 queues (~400 GB/s); 3-dim forms only hit ~250."""
        r0, nr = _band_rows(t)
        return bass.AP(xd, r0 * ROWSTRIDE,
                       [[ROWSTRIDE, nr], [CW, 8], [1, CW]])

    def row_ap(r):
        """DRAM AP for one image row across the 8 group partitions."""
        return bass.AP(xd, r * ROWSTRIDE, [[CW, 8], [1, CW]])

    def out_ap(dram, t):
        r0, nr = _band_rows(t)
        return bass.AP(dram.ap().tensor, r0 * ROWSTRIDE,
                       [[ROWSTRIDE, nr], [CW, 8], [1, CW]])

    with tile.TileContext(nc) as tc, ExitStack() as ctx:
        cpool = ctx.enter_context(tc.tile_pool(name="consts", bufs=1))
        xpool = ctx.enter_context(tc.tile_pool(name="x", bufs=1))
        wpool = ctx.enter_context(tc.tile_pool(name="w", bufs=1))
        tpool = ctx.enter_context(tc.tile_pool(name="tree", bufs=2))
        spool = ctx.enter_context(tc.tile_pool(name="stage", bufs=3))

        # ---- band loads (x ST loads first: queue position = land time) --
        BW = PAD + CW + PAD
        xball = xpool.tile([128, NBANDS * BW], bf16)
        xb = [xball[:, t * BW:(t + 1) * BW] for t in range(NBANDS)]
        # band 13 has unloaded partition rows; zero them (32-aligned base)
        # BEFORE its loads so the overlapping DMAs order after the memset
        nc.vector.memset(xb[13][64:128, :], 0.0)
        for t in range(NBANDS):
            r0, nr = _band_rows(t)
            eng = nc.sync if t % 2 == 0 else nc.scalar
            eng.dma_start(xb[t][0:8 * nr, PAD:PAD + CW], band_main_ap(t))
            # halo row above (reflect row 1 at the top edge)
            eng.dma_start(xb[t][112:120, PAD:PAD + CW],
                          row_ap(r0 - 1 if t > 0 else 1))
            # halo row below (reflect row 190 at the bottom edge); the E
            # wiring points i_src = nr at partitions 8*nr when nr < BR
            below = 120 if nr == BR else 8 * nr
            eng.dma_start(xb[t][below:below + 8, PAD:PAD + CW],
                          row_ap(r0 + nr if t < NBANDS - 1 else H - 2))

        # ---- consts (gpsimd queue) ----
        e_s = [cpool.tile([128, 128], bf16, name=f"epat{di}")
               for di in range(3)]
        hv_s = cpool.tile([8, 128], f32)
        gm_s = cpool.tile([128, 8], f32)
        at_s = cpool.tile([96, 576], f32)
        b_s = cpool.tile([72, 1], f32)
        r9_s = cpool.tile([72, 9], f32)
        g_s = cpool.tile([72, 8], f32)
        for s, d in ((b_s, b_d), (gm_s, gm_d), (at_s, at_d),
                     (e_s[0], e_d[0]), (e_s[1], e_d[1]), (e_s[2], e_d[2]),
                     (hv_s, hv_d), (r9_s, r9_d), (g_s, g_d)):
            nc.gpsimd.dma_start(s[:], d.ap())

        # pad + unused-partition init (avoid uninitialized reads by the
        # shifted matmul views); band 13 rows 80..112 never loaded
        for t in range(NBANDS):
            nc.vector.memset(xb[t][:, 0:PAD], 0.0)
            nc.vector.memset(xb[t][:, PAD + CW:], 0.0)

        # pre-load ACT spline tables off the weight-chain critical path
        exp_dummy = wpool.tile([72, 1], f32)
        nc.scalar.activation(exp_dummy[:], b_s[:],
                             mybir.ActivationFunctionType.Exp)

        # ---- pooled sums: per-band w-tree (DVE) + row-sum matmul ----
        wps_cm = tc.tile_pool(name="wpsum", bufs=1,
                              space=bass.MemorySpace.PSUM)
        wps = wps_cm.__enter__()
        pooled_p = wps.tile([96, 8], f32, tag="pooled")
        for t in range(NBANDS):
            _, nr = _band_rows(t)
            np_ = 8 * nr
            trA = tpool.tile([128, 768], bf16, tag="trA", name=f"trA{t}")
            trB = tpool.tile([128, 768], bf16, tag="trB", name=f"trB{t}")
            bsum = tpool.tile([128, 96], f32, tag="bsum", name=f"bs{t}")

            def v3(ap, wsz):
                return ap.rearrange("p (c w) -> p c w", w=wsz)

            xv = v3(xb[t][0:np_, PAD:PAD + CW], W)
            nc.vector.tensor_tensor(v3(trA[0:np_, 0:768], 96),
                                    xv[:, :, 0:96], xv[:, :, 96:192],
                                    op=mybir.AluOpType.add)
            a96 = v3(trA[0:np_, 0:768], 96)
            nc.vector.tensor_tensor(v3(trB[0:np_, 0:384], 48),
                                    a96[:, :, 0:48], a96[:, :, 48:96],
                                    op=mybir.AluOpType.add)
            b48 = v3(trB[0:np_, 0:384], 48)
            nc.vector.tensor_tensor(v3(trA[0:np_, 0:192], 24),
                                    b48[:, :, 0:24], b48[:, :, 24:48],
                                    op=mybir.AluOpType.add)
            a24 = v3(trA[0:np_, 0:192], 24)
            nc.vector.tensor_tensor(v3(bsum[0:np_, 0:96], 12),
                                    a24[:, :, 0:12], a24[:, :, 12:24],
                                    op=mybir.AluOpType.add)
            nc.tensor.matmul(pooled_p[:], bsum[0:np_, 0:96],
                             gm_s[0:np_, :],
                             start=(t == 0), stop=(t == NBANDS - 1))

        # ---- weight generation chain ----
        pooled_s = wpool.tile([96, 8], f32)
        nc.scalar.copy(pooled_s[:], pooled_p[:])
        lf_p = wps.tile([72, 1], f32, tag="lf")
        for g in range(8):
            nc.tensor.matmul(lf_p[:], at_s[:, 72 * g:72 * (g + 1)],
                             pooled_s[:, g:g + 1],
                             start=(g == 0), stop=(g == 7))
        e72 = wpool.tile([72, 1], f32)
        nc.scalar.activation(e72[:], lf_p[:],
                             mybir.ActivationFunctionType.Exp,
                             bias=b_s[:, 0:1], scale=1.0)
        rhsw = wpool.tile([72, 9], f32)
        nc.vector.tensor_scalar_mul(rhsw[:], r9_s[:], e72[:, 0:1])
        w89_p = wps.tile([8, 9], f32, tag="w89")
        nc.tensor.matmul(w89_p[:], g_s[:], rhsw[:])
        s8 = wpool.tile([8, 1], f32)
        nc.vector.tensor_reduce(s8[:], w89_p[:],
                                axis=mybir.AxisListType.X,
                                op=mybir.AluOpType.add)
        r8 = wpool.tile([8, 1], f32)
        nc.vector.reciprocal(r8[:], s8[:])
        w89s = wpool.tile([8, 9], f32)
        nc.vector.tensor_scalar_mul(w89s[:], w89_p[:], r8[:, 0:1])
        wbig_p = wps.tile([128, 9], f32, tag="wbig")
        nc.tensor.matmul(wbig_p[:], hv_s[:], w89s[:])
        wsc = wpool.tile([128, 9], f32)
        nc.scalar.copy(wsc[:], wbig_p[:])
        wps_cm.__exit__(None, None, None)

        # ---- the 3 block-banded stationaries: S_dj = sum_di E_di *
        # w[g(q), 3*di+dj] (per-partition row scaling; g(q) = q%8) ----
        S = [wpool.tile([128, 128], bf16, name=f"S{dj}") for dj in range(3)]
        for dj in range(3):
            nc.vector.tensor_scalar_mul(S[dj][:], e_s[0][:],
                                        wsc[:, dj:dj + 1])
            for di in (1, 2):
                nc.vector.scalar_tensor_tensor(
                    S[dj][:], e_s[di][:], wsc[:, 3 * di + dj:3 * di + dj + 1],
                    S[dj][:],
                    op0=mybir.AluOpType.mult, op1=mybir.AluOpType.add)

        # ---- main loop: one band at a time, acc = 4 PSUM banks
        # (3 x 512 main + edge-fix columns in bank 3) ----
        mpool = ctx.enter_context(
            tc.tile_pool(name="mpsum", bufs=2, space=bass.MemorySpace.PSUM))
        for t in range(NBANDS):
            _, nr = _band_rows(t)
            np_ = 8 * nr
            acc = mpool.tile([128, 2048], f32, tag="acc", name=f"acc{t}")
            djs = (0, 1, 2) if t % 2 == 0 else (2, 1, 0)
            for j, dj in enumerate(djs):
                first, last = (j == 0), (j == 2)
                for ch in range(3):
                    off = PAD + CH * ch + dj - 1
                    nc.tensor.matmul(acc[:, CH * ch:CH * (ch + 1)],
                                     S[dj][:], xb[t][:, off:off + CH],
                                     start=first, stop=last)
                wl = (1, 0, 1)[dj]
                wr = (190, 191, 190)[dj]
                ev = xb[t][:, PAD:PAD + CW].rearrange(
                    "p (c w) -> p c w", w=W)[:, :, wl:wr + 1:wr - wl]
                nc.tensor.matmul(
                    acc[:, 1536:1552].rearrange("p (c e) -> p c e", e=2),
                    S[dj][:], ev, start=first, stop=last)
            low_st = spool.tile([128, CW], bf16, tag="low")
            nc.scalar.copy(low_st[:], acc[:, 0:CW])
            nc.scalar.copy(
                low_st[:].rearrange("p (c w) -> p c w", w=W)[:, :, 0:W:W - 1],
                acc[:, 1536:1552].rearrange("p (c e) -> p c e", e=2))
            high_st = spool.tile([128, CW], bf16, tag="high")
            nc.vector.tensor_tensor(high_st[0:np_, :],
                                    xb[t][0:np_, PAD:PAD + CW],
                                    low_st[0:np_, :],
                                    op=mybir.AluOpType.subtract)
            nc.gpsimd.dma_start(out_ap(low_d, t), low_st[0:np_, :])
            nc.sync.dma_start(out_ap(high_d, t), high_st[0:np_, :])

    nc.compile()
    return nc


def _enable_ldw_opt():
    """All stationaries are fp32(r) (self-loading matmuls, no standalone
    16-bit LDWEIGHTS), so walrus's redundant-load-weight dedup is legal
    again; it lets same-stationary matmuls pipeline at ~N cycles."""
    import concourse.bass_utils as BU
    if getattr(BU, "_ldw_patched", False):
        return
    orig = BU.run_command

    def patched(cmd, *a, **kw):
        cmd = [c.replace("--enable-ldw-opt=false", "--enable-ldw-opt=true")
               if isinstance(c, str) else c for c in cmd]
        return orig(cmd, *a, **kw)

    BU.run_command = patched
    BU._ldw_patched = True


_nc_cache = None


def _get_program():
    global _nc_cache
    if _nc_cache is None:
        _nc_cache = _build_program()
    return _nc_cache


def _host_consts(conv_w, bn_gamma, bn_beta, bn_mean, bn_var):
    s_a = bn_gamma / np.sqrt(bn_var + BN_EPS)
    b72 = (bn_beta - bn_mean * s_a).astype(np.float32).reshape(72, 1)
    A = (conv_w * s_a[:, None]) / np.float32(H * W)   # (72, 64)

    # E wiring patterns: epat[di][q, p] = 1 iff q is the source partition
    # of output partition p for vertical tap di (halo rows at 112/120)
    epat = [np.zeros((128, 128), np.float32) for _ in range(3)]
    for p in range(128):
        i_out, g = p // 8, p % 8
        for di in range(3):
            i_src = i_out + di - 1
            if i_src == -1:
                q = 112 + g
            elif i_src == BR:
                q = 120 + g
            elif 0 <= i_src < BR:
                q = i_src * 8 + g
            else:
                continue
            epat[di][q, p] = 1.0

    epat = [e.astype(ml_dtypes.bfloat16) for e in epat]
    hv4 = (np.arange(8)[:, None] == (np.arange(128)[None, :] % 8)
           ).astype(np.float32)
    gmask = ((np.arange(128)[:, None] % 8) == np.arange(8)[None, :]
             ).astype(np.float32)
    at96 = np.zeros((96, 576), np.float32)
    for g in range(8):
        for cs in range(8):
            for w12 in range(12):
                at96[cs * 12 + w12, 72 * g:72 * (g + 1)] = A[:, 8 * g + cs]
    oc = np.arange(72)
    r9 = (oc[:, None] % 9 == np.arange(9)[None, :]).astype(np.float32)
    g728 = (oc[:, None] // 9 == np.arange(8)[None, :]).astype(np.float32)
    return dict(epat0=epat[0], epat1=epat[1], epat2=epat[2], hv4=hv4,
                gmask=gmask, at96=at96, b72=b72, r9=r9, g728=g728)


def _prep_inputs(x, conv_w, bn_gamma, bn_beta, bn_mean, bn_var):
    x = np.asarray(x, np.float32)
    consts = _host_consts(np.asarray(conv_w, np.float32),
                          np.asarray(bn_gamma, np.float32),
                          np.asarray(bn_beta, np.float32),
                          np.asarray(bn_mean, np.float32),
                          np.asarray(bn_var, np.float32))
    maps = []
    for i in range(N):
        xr = np.ascontiguousarray(np.transpose(x[i], (1, 0, 2))
                                  ).astype(ml_dtypes.bfloat16)
        maps.append(dict(x=xr, **consts))
    return maps


def _gather(res):
    low = np.stack([np.transpose(np.asarray(res[i]["low"]), (1, 0, 2))
                    for i in range(N)]).astype(np.float32)
    high = np.stack([np.transpose(np.asarray(res[i]["high"]), (1, 0, 2))
                     for i in range(N)]).astype(np.float32)
    return low, high


def kernel(x, conv_w, bn_gamma, bn_beta, bn_mean, bn_var):
    in_maps = _prep_inputs(x, conv_w, bn_gamma, bn_beta, bn_mean, bn_var)
    nc = _get_program()
    res = run_bass_kernel_spmd(nc, in_maps, list(range(N))).results
    return _gather(res)


if __name__ == "__main__":
    rng = np.random.default_rng(0)
    demo = dict(
        x=rng.standard_normal((N, IC, H, W), dtype=np.float32),
        conv_w=rng.standard_normal((72, 64)).astype(np.float32),
        bn_gamma=np.ones(72, np.float32),
        bn_beta=np.zeros(72, np.float32),
        bn_mean=rng.standard_normal(72).astype(np.float32) * 0.1,
        bn_var=rng.uniform(0.5, 1.5, 72).astype(np.float32),
    )
    low, high = kernel(**demo)
    print("ok", low.shape, high.shape)
